# revision 5
# baseline (speedup 1.0000x reference)
"""Trainium2 Bass kernel for nn_AttnLayer_60636348285536.

Computes o[b, c, n] = sum_{t,w,h} video[b,c,t,w,h] * f[n,w,h] / T,
returned as [B, C*N], where f are N=3 normalized Gaussian spatial
filters derived from tiny parameter vectors (computed on host and
replicated to every core).

Sharding: data-parallel over batch. B == 8 == n_cores, so each
NeuronCore reduces one [C, T, W, H] = [1024, 64, 14, 14] fp32 shard
(51.4 MB) — purely memory-bound streaming reduction, no collectives.

Per-core plan (per 128-channel tile, 8 tiles):
  - 1 HWDGE load of the full [128, 64*196] tile (6.4 MB, best DMA
    efficiency). (DMA accum_op=add was tried and is numerically broken
    on this path for large transfers: the CCE read-modify-write reads
    stale destination data beyond the first ~32 elements per row.)
  - DVE binary-tree adds over the 64 time slices -> [128, 196].
  - 3 fused scalar_tensor_tensor ops against the replicated filters
    (1/T folded in on host) -> [128, 3]; small DMA to DRAM.
"""

import numpy as np

B, C, T, W, H, N = 8, 1024, 64, 14, 14, 3
WH = W * H            # 196
NCORES = 8
CT = 128              # channels per SBUF tile (partition dim)
NTILE = C // CT       # 8 channel tiles per core
FREE = T * WH         # free-dim elements per partition after load (12544)

EPS_SIGMA = 1e-6
EPS_NORM = 1e-6

_cache = {}


def _filters_host(mu_x, mu_y, sigma_x, sigma_y):
    """Mirror of reference._filters in float64 numpy, scaled by 1/T."""
    mu_x = np.tanh(mu_x.astype(np.float64))
    mu_y = np.tanh(mu_y.astype(np.float64))
    sx = np.exp(1.5 - 2.0 / (1.0 + np.exp(-sigma_x.astype(np.float64))))
    sy = np.exp(1.5 - 2.0 / (1.0 + np.exp(-sigma_y.astype(np.float64))))
    inv_x = 1.0 / (sx**2 + EPS_SIGMA)
    inv_y = 1.0 / (sy**2 + EPS_SIGMA)
    mux = (W - 1) * ((mu_x + 1.0) / 2.0)
    muy = (H - 1) * ((mu_y + 1.0) / 2.0)
    xs = np.arange(W, dtype=np.float64)
    ys = np.arange(H, dtype=np.float64)
    dx = xs[None, :, None] - mux[:, None, None]          # [N, W, 1]
    dy = ys[None, None, :] - muy[:, None, None]          # [N, 1, H]
    quad = dx**2 * inv_x[:, None, None] + dy**2 * inv_y[:, None, None]
    f = np.exp(-0.5 * quad)                              # [N, W, H]
    f = f / (f.sum(axis=(1, 2), keepdims=True) + EPS_NORM)
    return (f / T).astype(np.float32)                    # fold the time mean


def _build():
    import concourse.bacc as bacc
    import concourse.mybir as mybir
    import concourse.tile as tile

    f32 = mybir.dt.float32
    nc = bacc.Bacc("TRN2", target_bir_lowering=False, debug=False,
                   num_devices=NCORES)
    video = nc.dram_tensor("video", [C, T * WH], f32, kind="ExternalInput").ap()
    frep = nc.dram_tensor("frep", [CT, N * WH], f32, kind="ExternalInput").ap()
    out = nc.dram_tensor("out", [C, N], f32, kind="ExternalOutput").ap()

    with tile.TileContext(nc) as tc:
        with tc.tile_pool(name="const", bufs=1) as cpool, \
             tc.tile_pool(name="data", bufs=3) as dpool, \
             tc.tile_pool(name="small", bufs=4) as spool:
            frep_sb = cpool.tile([CT, N * WH], f32)
            nc.sync.dma_start(out=frep_sb[:], in_=frep[:])

            for ti in range(NTILE):
                src = video[ti * CT:(ti + 1) * CT, :]
                buf = dpool.tile([CT, FREE], f32)
                nc.sync.dma_start(out=buf[:], in_=src[:])

                w = FREE
                while w > WH:
                    h = w // 2
                    nc.vector.tensor_add(out=buf[:, :h], in0=buf[:, :h],
                                         in1=buf[:, h:w])
                    w = h

                prod = spool.tile([CT, WH], f32, tag="prod")
                osb = spool.tile([CT, N], f32, tag="osb")
                for n in range(N):
                    nc.vector.scalar_tensor_tensor(
                        out=prod[:],
                        in0=buf[:, :WH],
                        scalar=1.0,
                        in1=frep_sb[:, n * WH:(n + 1) * WH],
                        op0=mybir.AluOpType.mult,
                        op1=mybir.AluOpType.mult,
                        accum_out=osb[:, n:n + 1])
                nc.sync.dma_start(out=out[ti * CT:(ti + 1) * CT, :], in_=osb[:])
    nc.compile()
    return nc


def _get_nc():
    if "nc" not in _cache:
        _cache["nc"] = _build()
    return _cache["nc"]


def kernel(video, mu_x, mu_y, sigma_x, sigma_y, _trace=False, _trace_kwargs=None):
    from concourse.bass_utils import run_bass_kernel_spmd

    nc = _get_nc()
    f = _filters_host(mu_x, mu_y, sigma_x, sigma_y)      # [N, W, H], /T folded
    frep = np.ascontiguousarray(
        np.broadcast_to(f.reshape(1, N * WH), (CT, N * WH)), dtype=np.float32)
    vid = np.ascontiguousarray(video, dtype=np.float32).reshape(B, C, T * WH)
    in_maps = [{"video": vid[b], "frep": frep} for b in range(NCORES)]
    kwargs = {}
    if _trace:
        kwargs = {"trace": True, "trace_kwargs": _trace_kwargs or {}}
    res = run_bass_kernel_spmd(nc, in_maps, core_ids=list(range(NCORES)),
                               **kwargs)
    if _trace:
        _cache["last_results"] = res
    outs = [res.results[b]["out"] for b in range(NCORES)]  # each [C, N]
    return np.stack(outs).reshape(B, C * N)


# revision 6
# speedup vs baseline: 1.2788x; 1.2788x over previous
"""Trainium2 Bass kernel for nn_AttnLayer_60636348285536.

Computes o[b, c, n] = sum_{t,w,h} video[b,c,t,w,h] * f[n,w,h] / T,
returned as [B, C*N], where f are N=3 normalized Gaussian spatial
filters derived from tiny parameter vectors (computed on host and
replicated to every core).

Sharding: data-parallel over batch. B == 8 == n_cores, so each
NeuronCore reduces one [C, T, W, H] = [1024, 64, 14, 14] fp32 shard
(51.4 MB) — purely memory-bound streaming reduction, no collectives.

Per-core plan (per 128-channel tile, 8 tiles):
  - 1 HWDGE load of the full [128, 64*196] tile (6.4 MB, best DMA
    efficiency). (DMA accum_op=add was tried and is numerically broken
    on this path for large transfers: the CCE read-modify-write reads
    stale destination data beyond the first ~32 elements per row.)
  - DVE binary-tree adds over the 64 time slices -> [128, 196].
  - 3 fused scalar_tensor_tensor ops against the replicated filters
    (1/T folded in on host) -> [128, 3]; small DMA to DRAM.
"""

import numpy as np

B, C, T, W, H, N = 8, 1024, 64, 14, 14, 3
WH = W * H            # 196
NCORES = 8
CT = 128              # channels per SBUF tile (partition dim)
NTILE = C // CT       # 8 channel tiles per core
FREE = T * WH         # free-dim elements per partition after load (12544)

EPS_SIGMA = 1e-6
EPS_NORM = 1e-6

_cache = {}


def _filters_host(mu_x, mu_y, sigma_x, sigma_y):
    """Mirror of reference._filters in float64 numpy, scaled by 1/T."""
    mu_x = np.tanh(mu_x.astype(np.float64))
    mu_y = np.tanh(mu_y.astype(np.float64))
    sx = np.exp(1.5 - 2.0 / (1.0 + np.exp(-sigma_x.astype(np.float64))))
    sy = np.exp(1.5 - 2.0 / (1.0 + np.exp(-sigma_y.astype(np.float64))))
    inv_x = 1.0 / (sx**2 + EPS_SIGMA)
    inv_y = 1.0 / (sy**2 + EPS_SIGMA)
    mux = (W - 1) * ((mu_x + 1.0) / 2.0)
    muy = (H - 1) * ((mu_y + 1.0) / 2.0)
    xs = np.arange(W, dtype=np.float64)
    ys = np.arange(H, dtype=np.float64)
    dx = xs[None, :, None] - mux[:, None, None]          # [N, W, 1]
    dy = ys[None, None, :] - muy[:, None, None]          # [N, 1, H]
    quad = dx**2 * inv_x[:, None, None] + dy**2 * inv_y[:, None, None]
    f = np.exp(-0.5 * quad)                              # [N, W, H]
    f = f / (f.sum(axis=(1, 2), keepdims=True) + EPS_NORM)
    return (f / T).astype(np.float32)                    # fold the time mean


def _build():
    import concourse.bacc as bacc
    import concourse.mybir as mybir
    import concourse.tile as tile

    f32 = mybir.dt.float32
    nc = bacc.Bacc("TRN2", target_bir_lowering=False, debug=False,
                   num_devices=NCORES)
    video = nc.dram_tensor("video", [C, T * WH], f32, kind="ExternalInput").ap()
    frep = nc.dram_tensor("frep", [CT, N * WH], f32, kind="ExternalInput").ap()
    out = nc.dram_tensor("out", [C, N], f32, kind="ExternalOutput").ap()

    HFREE = FREE // 2         # 6272 elements = 32 time slices
    with tile.TileContext(nc) as tc:
        with tc.tile_pool(name="const", bufs=1) as cpool, \
             tc.tile_pool(name="data", bufs=6) as dpool, \
             tc.tile_pool(name="small", bufs=4) as spool:
            frep_sb = cpool.tile([CT, N * WH], f32)
            nc.gpsimd.dma_start(out=frep_sb[:], in_=frep[:])

            ring = [nc.sync, nc.scalar]
            for ti in range(NTILE):
                src = video[ti * CT:(ti + 1) * CT, :]
                halves = []
                for hi in range(2):
                    buf = dpool.tile([CT, HFREE], f32, tag="data")
                    ring[(2 * ti + hi) % 2].dma_start(
                        out=buf[:], in_=src[:, hi * HFREE:(hi + 1) * HFREE])
                    w = HFREE
                    while w > WH:
                        h = w // 2
                        nc.vector.tensor_add(out=buf[:, :h], in0=buf[:, :h],
                                             in1=buf[:, h:w])
                        w = h
                    halves.append(buf)
                h0, h1 = halves
                nc.vector.tensor_add(out=h0[:, :WH], in0=h0[:, :WH],
                                     in1=h1[:, :WH])

                prod = spool.tile([CT, WH], f32, tag="prod")
                osb = spool.tile([CT, N], f32, tag="osb")
                for n in range(N):
                    nc.vector.scalar_tensor_tensor(
                        out=prod[:],
                        in0=h0[:, :WH],
                        scalar=1.0,
                        in1=frep_sb[:, n * WH:(n + 1) * WH],
                        op0=mybir.AluOpType.mult,
                        op1=mybir.AluOpType.mult,
                        accum_out=osb[:, n:n + 1])
                nc.gpsimd.dma_start(out=out[ti * CT:(ti + 1) * CT, :], in_=osb[:])
    nc.compile()
    return nc


def _get_nc():
    if "nc" not in _cache:
        _cache["nc"] = _build()
    return _cache["nc"]


def kernel(video, mu_x, mu_y, sigma_x, sigma_y, _trace=False, _trace_kwargs=None):
    from concourse.bass_utils import run_bass_kernel_spmd

    nc = _get_nc()
    f = _filters_host(mu_x, mu_y, sigma_x, sigma_y)      # [N, W, H], /T folded
    frep = np.ascontiguousarray(
        np.broadcast_to(f.reshape(1, N * WH), (CT, N * WH)), dtype=np.float32)
    vid = np.ascontiguousarray(video, dtype=np.float32).reshape(B, C, T * WH)
    in_maps = [{"video": vid[b], "frep": frep} for b in range(NCORES)]
    kwargs = {}
    if _trace:
        kwargs = {"trace": True, "trace_kwargs": _trace_kwargs or {}}
    res = run_bass_kernel_spmd(nc, in_maps, core_ids=list(range(NCORES)),
                               **kwargs)
    if _trace:
        _cache["last_results"] = res
    outs = [res.results[b]["out"] for b in range(NCORES)]  # each [C, N]
    return np.stack(outs).reshape(B, C * N)
